# revision 15
# baseline (speedup 1.0000x reference)
"""Trainium2 Bass kernel for DatasetIndexedTopK (streaming top-k retrieval).

Problem: scores = Q @ C^T with Q [512, 128], C [1M, 128]; return per-query
top-100 (scores, ids), matching jax.lax.top_k semantics (ties -> lower id).

Design (per core, 8-way shard over candidates):
  - host pre-transposes queries -> qT [128, 512] and this core's candidate
    shard -> candT [128, 131072] so the contraction dim D=128 sits on SBUF
    partitions for the PE.
  - stream candT in 8192-wide SBUF tiles; for each 128-query chunk, fp32
    matmuls (N=512 each, exact fp32 so selection matches the reference) fill
    [128, 2048] PSUM tiles, which the scalar engine stages into [128, 4096]
    SBUF spans (PE/ACT/DVE pipeline, DVE is the bottleneck engine).
  - DVE InstMax extracts each 4096-span's per-query top-8 values into a
    summary array S [128, 256/chunk]; DVE max_index recovers the 8
    within-span positions into P8 (uint32).  A span holding >8 of the
    global top-100 for some query has probability ~1e-6 -- ignored (deep
    local ranks ~48+ may differ, but only local ranks <~35 can ever reach
    the global top-100).
  - per chunk, 13 rounds of (max, max_index, match_replace) over S extract
    the per-core top-104 values plus their S-positions.
  - host decodes S-position -> (block, within-block position) -> global
    candidate position, then merges the 8 cores' top-104 lists exactly
    (stable by (-score, id), matching the reference's streaming tie-breaks).
"""

import numpy as np

P = 128                 # SBUF partitions / queries per chunk
D = 128                 # embedding dim (contraction)
Q = 512                 # queries
NCORES = 8
NCAND_TOTAL = 256 * 4096
NCAND = NCAND_TOTAL // NCORES    # 131072 candidates per core
CTILE = 8192            # candidate columns per DMA tile
PBLK = 2048             # columns per PSUM tile (4 banks)
BLK = 4096              # columns per InstMax block (SBUF staged)
NBLK = NCAND // BLK     # 32 summary blocks per chunk
S_W = NBLK * 8          # summary width per chunk (256)
NCHUNK = Q // P         # 4 query chunks
KEXT = 104              # values extracted per core per query (13 rounds x 8)
NROUNDS = KEXT // 8
FLOAT_MIN = float(np.finfo(np.float32).min)

_CACHE = {}


def _build_bass(repeat=1, do_max=True, do_idx=True, ring_bufs=4, cand_bufs=3,
                pblk=1024, psum_bufs=4):
    import concourse.bacc as bacc
    import concourse.mybir as mybir
    from concourse.tile import TileContext
    from contextlib import ExitStack

    f32 = mybir.dt.float32
    u32 = mybir.dt.uint32

    nc = bacc.Bacc()
    qT = nc.declare_dram_parameter("qT", [D, Q], f32, isOutput=False)
    candT = nc.declare_dram_parameter("candT", [D, NCAND], f32, isOutput=False)
    out_val = nc.declare_dram_parameter("out_val", [Q, KEXT], f32, isOutput=True)
    out_spos = nc.declare_dram_parameter("out_spos", [Q, KEXT], u32, isOutput=True)
    out_p8 = nc.declare_dram_parameter("out_p8", [Q, S_W], u32, isOutput=True)

    with ExitStack() as ctx:
        tc = ctx.enter_context(TileContext(nc))
        qpool = ctx.enter_context(tc.tile_pool(name="q", bufs=1))
        cpool = ctx.enter_context(tc.tile_pool(name="cand", bufs=cand_bufs))
        pspool = ctx.enter_context(tc.tile_pool(name="ps", bufs=psum_bufs, space="PSUM"))
        ring = ctx.enter_context(tc.tile_pool(name="ring", bufs=ring_bufs))
        acc = ctx.enter_context(tc.tile_pool(name="acc", bufs=1))
        outp = ctx.enter_context(tc.tile_pool(name="outp", bufs=2))

        qsb = qpool.tile([D, Q], f32, tag="qsb")
        nc.sync.dma_start(qsb[:], qT[:])

        S_all = acc.tile([P, NCHUNK * S_W], f32, tag="S")
        P8_all = acc.tile([P, NCHUNK * S_W], u32, tag="P8")

        for t in range(repeat * (NCAND // CTILE)):
            t = t % (NCAND // CTILE)
            ct = cpool.tile([D, CTILE], f32, tag="cand")
            nc.sync.dma_start(ct[:], candT[:, t * CTILE:(t + 1) * CTILE])
            for qc in range(NCHUNK):
                for blk in range(CTILE // BLK):
                    rt = ring.tile([P, BLK], f32, tag="ring")
                    for sub in range(BLK // pblk):
                        ps = pspool.tile([P, pblk], f32, tag="ps")
                        for j in range(pblk // 512):
                            col = blk * BLK + sub * pblk + j * 512
                            nc.tensor.matmul(
                                ps[:, j * 512:(j + 1) * 512],
                                lhsT=qsb[:, qc * P:(qc + 1) * P],
                                rhs=ct[:, col: col + 512],
                                start=True,
                                stop=True,
                            )
                        # stage to SBUF on the (otherwise idle) scalar engine
                        nc.scalar.copy(
                            rt[:, sub * pblk:(sub + 1) * pblk], ps[:]
                        )
                    g = t * (CTILE // BLK) + blk     # global block index
                    so = qc * S_W + g * 8
                    if do_max:
                        nc.vector.max(out=S_all[:, so:so + 8], in_=rt[:])
                    else:
                        nc.vector.memset(S_all[:, so:so + 8], 0.0)
                    if do_idx:
                        nc.vector.max_index(
                            out=P8_all[:, so:so + 8],
                            in_max=S_all[:, so:so + 8],
                            in_values=rt[:],
                        )
                    elif t == 0:
                        nc.vector.memset(P8_all[:, so:so + 8], 0)

        for qc in range(NCHUNK):
            cur = S_all[:, qc * S_W:(qc + 1) * S_W]
            wval = outp.tile([P, KEXT], f32, tag="wval")
            wpos = outp.tile([P, KEXT], u32, tag="wpos")
            for r in range(NROUNDS):
                m8 = wval[:, r * 8:(r + 1) * 8]
                nc.vector.max(out=m8, in_=cur)
                nc.vector.max_index(
                    out=wpos[:, r * 8:(r + 1) * 8], in_max=m8, in_values=cur
                )
                nc.vector.match_replace(
                    out=cur, in_to_replace=m8, in_values=cur, imm_value=FLOAT_MIN
                )
            nc.sync.dma_start(out_val[qc * P:(qc + 1) * P, :], wval[:])
            nc.sync.dma_start(out_spos[qc * P:(qc + 1) * P, :], wpos[:])
            nc.sync.dma_start(
                out_p8[qc * P:(qc + 1) * P, :], P8_all[:, qc * S_W:(qc + 1) * S_W]
            )
    nc.compile()
    return nc


def _get_bass():
    if "nc" not in _CACHE:
        _CACHE["nc"] = _build_bass()
    return _CACHE["nc"]


def kernel(query_embeddings, candidate_embeddings, candidate_indices, k):
    from concourse.bass_utils import run_bass_kernel_spmd

    q = np.ascontiguousarray(np.asarray(query_embeddings, dtype=np.float32))
    c = np.asarray(candidate_embeddings, dtype=np.float32).reshape(NCAND_TOTAL, D)
    ids_flat = np.asarray(candidate_indices).reshape(-1)
    k = int(k)
    assert k <= KEXT

    qT = np.ascontiguousarray(q.T)                       # [128, 512]
    in_maps = []
    for core in range(NCORES):
        shard = c[core * NCAND:(core + 1) * NCAND]       # [131072, 128]
        in_maps.append({
            "qT": qT,
            "candT": np.ascontiguousarray(shard.T),      # [128, 131072]
        })

    nc = _get_bass()
    res = run_bass_kernel_spmd(nc, in_maps, core_ids=list(range(NCORES))).results

    # ---- host decode + exact 8-way merge ----
    all_vals = np.empty((Q, NCORES * KEXT), dtype=np.float32)
    all_gpos = np.empty((Q, NCORES * KEXT), dtype=np.int64)
    for core in range(NCORES):
        val = res[core]["out_val"]                       # [512, 104] f32
        spos = res[core]["out_spos"].astype(np.int64)    # [512, 104]
        p8 = res[core]["out_p8"].astype(np.int64)        # [512, 512]
        within = np.take_along_axis(p8, spos, axis=1)    # [512, 104]
        lpos = (spos >> 3) * BLK + within
        all_vals[:, core * KEXT:(core + 1) * KEXT] = val
        all_gpos[:, core * KEXT:(core + 1) * KEXT] = core * NCAND + lpos

    out_scores = np.empty((Q, k), dtype=np.float32)
    out_pos = np.empty((Q, k), dtype=np.int64)
    for qi in range(Q):
        order = np.lexsort((all_gpos[qi], -all_vals[qi]))[:k]
        out_scores[qi] = all_vals[qi, order]
        out_pos[qi] = all_gpos[qi, order]

    out_ids = ids_flat[out_pos].astype(ids_flat.dtype)
    return out_scores, out_ids
